# revision 2
# baseline (speedup 1.0000x reference)
"""Trainium2 Bass kernel for the DML prototype-matching head (retrieval_knn).

Math (reference):
    emb   = x / max(||x||_2 over E, 1e-12)            # [N, E, H, W]
    reps  = reps_raw / max(||reps_raw||_2 over E)     # [C, 1, E]
    dot   = einsum('nehw,cme->ncmhw', emb, reps)
    sq    = max(2 - 2*dot, 0)
    dist  = sqrt(sq)                                  # [N, C, 1, H, W]
    probs = exp(-sq / (2*0.5^2)) = exp(-2*sq)
    cls   = probs / sum_c probs                       # [N, C, H, W]
    returns (cls, dist, reps, emb)

Sharding: data-parallel over batch N=8 -> one batch element per NeuronCore.
The tiny prototype tensor is normalized + transposed on host and replicated.

Per-core device layout: x_n viewed as [E=256, HW=16384], E split into two
128-partition halves. Spatial axis processed in 8 tiles of F=2048 columns,
matmuls chunked at 512 columns (one PSUM bank).

sqrt/rsqrt are computed as exp(k*ln(x)) so that ACT stays on the single
`natural_log_exp_and_others` table set (no ~2.7us table reloads), and because
the Rsqrt/Reciprocal ACT functions are disallowed for accuracy reasons.
"""

import numpy as np

N, E, H, W = 8, 256, 128, 128
C = 81
HW = H * W
F = 2048          # spatial tile (columns)
NT = HW // F      # 8 tiles
CH = 512          # matmul chunk (one PSUM bank of fp32)
NCH = F // CH     # 4 chunks per tile
EPS = 1e-12

_prog = None


def _build_program():
    import concourse.bacc as bacc
    import concourse.mybir as mybir
    import concourse.tile as tile
    from concourse.alu_op_type import AluOpType

    fp32 = mybir.dt.float32
    nc = bacc.Bacc("TRN2", target_bir_lowering=False, debug=False, num_devices=8)

    x_in = nc.dram_tensor("x", [E, HW], fp32, kind="ExternalInput")
    rt_in = nc.dram_tensor("rt", [E, C], fp32, kind="ExternalInput")
    emb_out = nc.dram_tensor("emb", [E, HW], fp32, kind="ExternalOutput")
    dist_out = nc.dram_tensor("dist", [C, HW], fp32, kind="ExternalOutput")
    cls_out = nc.dram_tensor("cls", [C, HW], fp32, kind="ExternalOutput")

    x_r = x_in.rearrange("(b p) f -> p b f", b=2)
    emb_r = emb_out.rearrange("(b p) f -> p b f", b=2)
    rt_r = rt_in.rearrange("(b p) c -> p b c", b=2)

    # Register const APs for activation bias values (mimics Bass.__init__).
    for v in (EPS * EPS, 1e-30):
        t = nc.alloc_sbuf_tensor(f"const-float32-{v}", [128, 1], fp32)
        nc.gpsimd.memset(t.ap(), v)
        nc.const_aps.aps[(fp32, v)] = t.ap()
    nc.all_engine_barrier()

    with tile.TileContext(nc) as tc:
        with (
            tc.tile_pool(name="px", bufs=3) as px,
            tc.tile_pool(name="px2", bufs=2) as px2,
            tc.tile_pool(name="pbig", bufs=2) as pbig,
            tc.tile_pool(name="prow", bufs=2) as prow,
            tc.tile_pool(name="pconst", bufs=1) as pconst,
            tc.tile_pool(name="ppd", bufs=4, space="PSUM") as ppd,
            tc.tile_pool(name="pps", bufs=2, space="PSUM") as pps,
            tc.tile_pool(name="ppp", bufs=2, space="PSUM") as ppp,
        ):
            rt_sb = pconst.tile([128, 2, C], fp32)
            nc.sync.dma_start(rt_sb[:], rt_r[:])
            ones = pconst.tile([128, 1], fp32)
            nc.vector.memset(ones[:], 1.0)

            for t in range(NT):
                ts = slice(t * F, (t + 1) * F)
                xt = px.tile([128, 2, F], fp32)
                nc.sync.dma_start(xt[:], x_r[:, :, ts])

                ss_row = prow.tile([1, F], fp32, tag="ss")
                psds = []
                for c in range(NCH):
                    cs = slice(c * CH, (c + 1) * CH)
                    x2c = px2.tile([128, 2, CH], fp32, tag="x2")
                    nc.vector.tensor_tensor(
                        x2c[:], xt[:, :, cs], xt[:, :, cs], AluOpType.mult
                    )
                    psd = ppd.tile([C, CH], fp32, tag="psd")
                    nc.tensor.matmul(
                        psd[:], rt_sb[:, 0, :], xt[:, 0, cs], start=True, stop=False
                    )
                    nc.tensor.matmul(
                        psd[:], rt_sb[:, 1, :], xt[:, 1, cs], start=False, stop=True
                    )
                    psds.append(psd)
                    pss = pps.tile([1, CH], fp32, tag="pss")
                    nc.tensor.matmul(
                        pss[:], ones[:], x2c[:, 0, :], start=True, stop=False
                    )
                    nc.tensor.matmul(
                        pss[:], ones[:], x2c[:, 1, :], start=False, stop=True
                    )
                    nc.vector.tensor_copy(ss_row[:, cs], pss[:])

                # inv = (ss + eps^2) ^ -1/2, computed as exp(-0.5 * ln(.))
                nc.scalar.activation(
                    ss_row[:], ss_row[:], mybir.ActivationFunctionType.Ln, bias=EPS * EPS
                )
                nc.scalar.activation(
                    ss_row[:], ss_row[:], mybir.ActivationFunctionType.Exp, scale=-0.5
                )
                invb = pbig.tile([128, F], fp32, tag="invb")
                nc.gpsimd.partition_broadcast(invb[:], ss_row[:])

                # sq = max(2 - 2 * draw * inv, 0)
                sqt = pbig.tile([C, F], fp32, tag="sq")
                for c in range(NCH):
                    cs = slice(c * CH, (c + 1) * CH)
                    nc.vector.scalar_tensor_tensor(
                        sqt[:, cs],
                        psds[c][:],
                        -2.0,
                        invb[0:C, cs],
                        op0=AluOpType.mult,
                        op1=AluOpType.mult,
                    )
                nc.vector.tensor_scalar(
                    sqt[:], sqt[:], 2.0, 0.0, op0=AluOpType.add, op1=AluOpType.max
                )

                probst = pbig.tile([C, F], fp32, tag="probs")
                nc.scalar.activation(
                    probst[:], sqt[:], mybir.ActivationFunctionType.Exp, scale=-2.0
                )
                # dist = sqrt(sq) = exp(0.5 * ln(sq + tiny))
                distt = pbig.tile([C, F], fp32, tag="dist")
                nc.scalar.activation(
                    distt[:], sqt[:], mybir.ActivationFunctionType.Ln, bias=1e-30
                )
                nc.scalar.activation(
                    distt[:], distt[:], mybir.ActivationFunctionType.Exp, scale=0.5
                )

                pr_row = prow.tile([1, F], fp32, tag="pr")
                for c in range(NCH):
                    cs = slice(c * CH, (c + 1) * CH)
                    psp = ppp.tile([1, CH], fp32, tag="psp")
                    nc.tensor.matmul(psp[:], ones[0:C, :], probst[:, cs])
                    nc.vector.reciprocal(pr_row[:, cs], psp[:])
                prb = pbig.tile([C, F], fp32, tag="prb")
                nc.gpsimd.partition_broadcast(prb[:], pr_row[:])

                clst = pbig.tile([C, F], fp32, tag="cls")
                nc.vector.tensor_tensor(clst[:], probst[:], prb[:], AluOpType.mult)

                # emb in place over xt
                nc.vector.tensor_tensor(
                    xt[:, 0, :], xt[:, 0, :], invb[:], AluOpType.mult
                )
                nc.vector.tensor_tensor(
                    xt[:, 1, :], xt[:, 1, :], invb[:], AluOpType.mult
                )

                nc.sync.dma_start(emb_r[:, :, ts], xt[:])
                nc.sync.dma_start(dist_out[:, ts], distt[:])
                nc.sync.dma_start(cls_out[:, ts], clst[:])

    nc.compile()
    return nc


def _get_program():
    global _prog
    if _prog is None:
        _prog = _build_program()
    return _prog


def _host_reps(reps_raw):
    # exact reference math on host for the tiny prototype tensor
    nrm = np.linalg.norm(reps_raw.astype(np.float32), axis=2, keepdims=True)
    reps = reps_raw / np.clip(nrm, EPS, None)
    return reps.astype(np.float32)


def run(x, reps_raw, trace=False):
    from concourse.bass_utils import run_bass_kernel_spmd

    nc = _get_program()
    reps = _host_reps(reps_raw)
    rt = np.ascontiguousarray(reps.reshape(C, E).T)  # [E, C]
    in_maps = [
        {"x": np.ascontiguousarray(x[i].reshape(E, HW)), "rt": rt} for i in range(N)
    ]
    res = run_bass_kernel_spmd(nc, in_maps, list(range(N)), trace=trace)

    cls = np.empty((N, C, H, W), np.float32)
    dist = np.empty((N, C, 1, H, W), np.float32)
    emb = np.empty((N, E, H, W), np.float32)
    for i in range(N):
        r = res.results[i]
        cls[i] = r["cls"].reshape(C, H, W)
        dist[i, :, 0] = r["dist"].reshape(C, H, W)
        emb[i] = r["emb"].reshape(E, H, W)
    return (cls, dist, reps, emb), res


def kernel(x, reps_raw):
    (cls, dist, reps, emb), _ = run(np.asarray(x), np.asarray(reps_raw))
    return (cls, dist, reps, emb)


# revision 6
# speedup vs baseline: 1.3755x; 1.3755x over previous
"""Trainium2 Bass kernel for the DML prototype-matching head (retrieval_knn).

Math (reference):
    emb   = x / max(||x||_2 over E, 1e-12)            # [N, E, H, W]
    reps  = reps_raw / max(||reps_raw||_2 over E)     # [C, 1, E]
    dot   = einsum('nehw,cme->ncmhw', emb, reps)
    sq    = max(2 - 2*dot, 0)
    dist  = sqrt(sq)                                  # [N, C, 1, H, W]
    probs = exp(-2*sq)
    cls   = probs / sum_c probs                       # [N, C, H, W]
    returns (cls, dist, reps, emb)

Sharding: data-parallel over batch N=8 -> one batch element per NeuronCore.
The tiny prototype tensor is normalized + transposed on host and replicated.

Per-core device layout: x_n viewed as [E=256, HW=16384], E split into two
128-partition halves (xa, xb). Spatial axis processed in 8 tiles of F=2048
columns, matmuls chunked at 512 columns (one PSUM bank).

Key engine choices:
  - All matmuls use float32r (1 cycle/row vs fp32's 4).
  - sqrt/rsqrt go through exp(k*ln(x)) so ACT stays on the single
    `natural_log_exp_and_others` table set (no ~2.7us table reloads); the
    act_info.json passed to walrus is patched so `exp` resolves to that set.
  - 1/sum(probs) uses the single-instruction reciprocal_approx_fast custom
    DVE op (~51 ULP) instead of the ~3.3us iterative InstReciprocal.
"""

import json
import os
import tempfile

import numpy as np

N, E, H, W = 8, 256, 128, 128
C = 81
HW = H * W
F = 2048          # spatial tile (columns)
NT = HW // F      # 8 tiles
CH = 512          # matmul chunk (one PSUM bank of fp32)
NCH = F // CH     # 4 chunks per tile
EPS = 1e-12

_prog = None


def _patch_act_tables():
    """Point walrus at an act_info.json whose only exp-bearing set is
    `natural_log_exp_and_others`, so Ln/Exp sequences never reload tables."""
    if os.environ.get("BASS_ACT_ROOT_JSON_PATH"):
        return
    from neuronxcc.driver.Job import Job
    from neuronxcc.driver.jobs.support.FindActInfo import findActInfoFile

    src = findActInfoFile(Job.getPackageDir(), "gen3")
    src_dir = os.path.dirname(src)
    with open(src) as f:
        d = json.load(f)
    keep, moved = [], []
    for s in d["act_func_sets"]:
        if s["name"] in ("exp_and_others", "exp_and_friends"):
            continue
        (moved if s["name"] == "natural_log_exp_and_others" else keep).append(s)
    d["act_func_sets"] = moved + keep
    patched_dir = tempfile.mkdtemp(prefix="act_tables_")
    for fn in os.listdir(src_dir):
        if fn != "act_info.json":
            os.symlink(os.path.join(src_dir, fn), os.path.join(patched_dir, fn))
    patched = os.path.join(patched_dir, "act_info.json")
    with open(patched, "w") as f:
        json.dump(d, f)
    os.environ["BASS_ACT_ROOT_JSON_PATH"] = patched


def _build_program():
    import concourse.bacc as bacc
    import concourse.mybir as mybir
    import concourse.tile as tile
    from concourse.alu_op_type import AluOpType

    _patch_act_tables()

    fp32 = mybir.dt.float32
    f32r = mybir.dt.float32r
    Ln = mybir.ActivationFunctionType.Ln
    Exp = mybir.ActivationFunctionType.Exp
    nc = bacc.Bacc("TRN2", target_bir_lowering=False, debug=False, num_devices=8)

    x_in = nc.dram_tensor("x", [E, HW], fp32, kind="ExternalInput")
    rt_in = nc.dram_tensor("rt", [E, C], fp32, kind="ExternalInput")
    emb_out = nc.dram_tensor("emb", [E, HW], fp32, kind="ExternalOutput")
    dist_out = nc.dram_tensor("dist", [C, HW], fp32, kind="ExternalOutput")
    cls_out = nc.dram_tensor("cls", [C, HW], fp32, kind="ExternalOutput")

    rt_r = rt_in.rearrange("(b p) c -> p b c", b=2)

    # Register const APs for activation bias values (mimics Bass.__init__).
    for v in (EPS * EPS, 1e-30):
        t = nc.alloc_sbuf_tensor(f"const-float32-{v}", [128, 1], fp32)
        nc.gpsimd.memset(t.ap(), v)
        nc.const_aps.aps[(fp32, v)] = t.ap()
    nc.all_engine_barrier()

    with tile.TileContext(nc) as tc:
        with (
            tc.tile_pool(name="px", bufs=4) as px,
            tc.tile_pool(name="px2", bufs=2) as px2,
            tc.tile_pool(name="pemb", bufs=4) as pemb,
            tc.tile_pool(name="pbig", bufs=2) as pbig,
            tc.tile_pool(name="prow", bufs=1) as prow,
            tc.tile_pool(name="pconst", bufs=1) as pconst,
            tc.tile_pool(name="ppd", bufs=3, space="PSUM") as ppd,
            tc.tile_pool(name="pps", bufs=2, space="PSUM") as pps,
            tc.tile_pool(name="ppp", bufs=2, space="PSUM") as ppp,
        ):
            # Prototypes + ones vector as float32r (rounded on the way in:
            # gpsimd cast-DMA / DVE cast-copy, as the BIR verifier requires).
            rt_sb = pconst.tile([128, 2, C], f32r)
            nc.gpsimd.dma_start(rt_sb[:], rt_r[:])
            ones = pconst.tile([128, 1], fp32)
            nc.vector.memset(ones[:], 1.0)
            ones_r = pconst.tile([128, 1], f32r)
            nc.vector.tensor_copy(ones_r[:], ones[:])
            rta = rt_sb[:, 0, :]
            rtb = rt_sb[:, 1, :]

            for t in range(NT):
                ts = slice(t * F, (t + 1) * F)
                # x halves rounded to f32r at load (cast DMA must be gpsimd)
                xa = px.tile([128, F], f32r, tag="x")
                xb = px.tile([128, F], f32r, tag="x")
                nc.gpsimd.dma_start(xa[:], x_in[0:128, ts])
                nc.gpsimd.dma_start(xb[:], x_in[128:256, ts])
                x2a = px2.tile([128, F], f32r, tag="x2")
                x2b = px2.tile([128, F], f32r, tag="x2")
                nc.vector.tensor_tensor(
                    x2a[:], xa[:].bitcast(fp32), xa[:].bitcast(fp32), AluOpType.mult
                )
                nc.vector.tensor_tensor(
                    x2b[:], xb[:].bitcast(fp32), xb[:].bitcast(fp32), AluOpType.mult
                )

                ss_row = prow.tile([1, F], fp32, tag="ss", bufs=2)
                psds = []
                for c in range(NCH):
                    cs = slice(c * CH, (c + 1) * CH)
                    psd = ppd.tile([C, CH], fp32, tag="psd")
                    nc.tensor.matmul(psd[:], rta, xa[:, cs], start=True, stop=False)
                    nc.tensor.matmul(psd[:], rtb, xb[:, cs], start=False, stop=True)
                    psds.append(psd)
                    pss = pps.tile([1, CH], fp32, tag="pss")
                    nc.tensor.matmul(
                        pss[:], ones_r[:], x2a[:, cs], start=True, stop=False
                    )
                    nc.tensor.matmul(
                        pss[:], ones_r[:], x2b[:, cs], start=False, stop=True
                    )
                    # ln(ss + eps^2) straight out of PSUM, per chunk
                    nc.scalar.activation(ss_row[:, cs], pss[:], Ln, bias=EPS * EPS)
                # inv = exp(-0.5 * ln(ss)) ; broadcast to all partitions
                nc.scalar.activation(ss_row[:], ss_row[:], Exp, scale=-0.5)
                invb = pbig.tile([128, F], fp32, tag="invb")
                nc.gpsimd.partition_broadcast(invb[:], ss_row[:])

                # sq = max(2 - 2 * draw * inv, 0)
                sqt = pbig.tile([C, F], fp32, tag="sq")
                for c in range(NCH):
                    cs = slice(c * CH, (c + 1) * CH)
                    nc.vector.scalar_tensor_tensor(
                        sqt[:, cs],
                        psds[c][:],
                        -2.0,
                        invb[0:C, cs],
                        op0=AluOpType.mult,
                        op1=AluOpType.mult,
                    )
                nc.vector.tensor_scalar(
                    sqt[:], sqt[:], 2.0, 0.0, op0=AluOpType.add, op1=AluOpType.max
                )

                # probs as f32r so the probs-sum matmul stays on the fast path
                probst = pbig.tile([C, F], f32r, tag="probs")
                nc.scalar.activation(probst[:], sqt[:], Exp, scale=-2.0)
                # dist = sqrt(sq) = exp(0.5 * ln(sq + tiny))
                distt = pbig.tile([C, F], fp32, tag="dist")
                nc.scalar.activation(distt[:], sqt[:], Ln, bias=1e-30)
                nc.scalar.activation(distt[:], distt[:], Exp, scale=0.5)

                pr_row = prow.tile([1, F], fp32, tag="pr", bufs=1)
                for c in range(NCH):
                    cs = slice(c * CH, (c + 1) * CH)
                    psp = ppp.tile([1, CH], fp32, tag="psp")
                    nc.tensor.matmul(psp[:], ones_r[0:C, :], probst[:, cs])
                    nc.vector.reciprocal_approx_fast(pr_row[:, cs], psp[:])
                prb = pbig.tile([C, F], fp32, tag="invb")
                nc.gpsimd.partition_broadcast(prb[:], pr_row[:])

                clst = pbig.tile([C, F], fp32, tag="sq")
                nc.vector.tensor_tensor(
                    clst[:], probst[:].bitcast(fp32), prb[:], AluOpType.mult
                )

                emb_a = pemb.tile([128, F], fp32, tag="emb")
                emb_b = pemb.tile([128, F], fp32, tag="emb")
                nc.vector.tensor_tensor(
                    emb_a[:], xa[:].bitcast(fp32), invb[:], AluOpType.mult
                )
                nc.vector.tensor_tensor(
                    emb_b[:], xb[:].bitcast(fp32), invb[:], AluOpType.mult
                )

                nc.sync.dma_start(emb_out[0:128, ts], emb_a[:])
                nc.sync.dma_start(emb_out[128:256, ts], emb_b[:])
                nc.sync.dma_start(dist_out[:, ts], distt[:])
                nc.sync.dma_start(cls_out[:, ts], clst[:])

    nc.compile()
    return nc


def _get_program():
    global _prog
    if _prog is None:
        _prog = _build_program()
    return _prog


def _host_reps(reps_raw):
    # exact reference math on host for the tiny prototype tensor
    nrm = np.linalg.norm(reps_raw.astype(np.float32), axis=2, keepdims=True)
    reps = reps_raw / np.clip(nrm, EPS, None)
    return reps.astype(np.float32)


def run(x, reps_raw, trace=False):
    from concourse.bass_utils import run_bass_kernel_spmd

    nc = _get_program()
    reps = _host_reps(reps_raw)
    rt = np.ascontiguousarray(reps.reshape(C, E).T)  # [E, C]
    in_maps = [
        {"x": np.ascontiguousarray(x[i].reshape(E, HW)), "rt": rt} for i in range(N)
    ]
    res = run_bass_kernel_spmd(nc, in_maps, list(range(N)), trace=trace)

    cls = np.empty((N, C, H, W), np.float32)
    dist = np.empty((N, C, 1, H, W), np.float32)
    emb = np.empty((N, E, H, W), np.float32)
    for i in range(N):
        r = res.results[i]
        cls[i] = r["cls"].reshape(C, H, W)
        dist[i, :, 0] = r["dist"].reshape(C, H, W)
        emb[i] = r["emb"].reshape(E, H, W)
    return (cls, dist, reps, emb), res


def kernel(x, reps_raw):
    (cls, dist, reps, emb), _ = run(np.asarray(x), np.asarray(reps_raw))
    return (cls, dist, reps, emb)
